# revision 38
# baseline (speedup 1.0000x reference)
"""AVSL similarity kernel for Trainium2 (8 NeuronCores, data-parallel over B1).

Math (per (b1,b2) pair, d-vector chain over 3 layers):
  n_l = (normalize(emb1_l[b1]) - normalize(emb2_l[b2]))**2        [D]
  hat_0 = n_0
  hat_l = (1-P_l) * (hat_{l-1} @ W_l) + P_l * n_l,  l=1,2
  P_l   = sigmoid(alpha_l * cert1_l[b1] * cert2_l[b2] + beta_l)
  W_l   = col-top3-masked, col-normalized link_{l-1}
  out[b1,b2] = sum_d hat_2

Device decomposition, [d(=128 partitions), b2(=512 free)] layout, Q_l = 1-P_l
(sigmoid of negated argument):
  A  = n1 - W1^T n0                       (PE: negated weights + I*n1)
  v1 = Q1 * A          => hat1 = n1 - v1  (DVE; Q via ACT)
  B  = n2 - W2^T n1 + W2^T v1  (= n2 - mm2)   (PE)
  v2 = Q2 * B          => hat2 = n2 - v2  (DVE)
  out_row = 1^T n2 - 1^T v2               (PE M=1 matmuls, 4 rows per PSUM bank
                                           at partitions 0/32/64/96)
Rows processed in pairs: A/B PSUM tiles and Q/v tiles span two rows
([128,1024], 2 PSUM banks) so the PSUM-read multiplies amortize overhead.
Engine split: ACT: n1, Q1, Q2 (+ n2 on most rows) | DVE: n0 (sub+sq), v1, v2,
out-copy (+ n2 on some rows) | PE: 7 matmuls/row. Matmul operands bf16.

Sharding: emb1/cert1 rows split 64/core; emb2/cert2/links/alpha/beta replicated.
"""
import os
import sys

sys.path.insert(0, "/opt/trn_rl_repo")

import numpy as np

import concourse.bass as bass
import concourse.tile as tile
from concourse import bacc, mybir
from concourse.bass_utils import run_bass_kernel_spmd

N_CORES = 8
B1, B2, D = 512, 512, 128
RPC = B1 // N_CORES  # rows of ovr_sim per core
F32 = mybir.dt.float32
BF16 = mybir.dt.bfloat16
AF = mybir.ActivationFunctionType
OP = mybir.AluOpType
AX = mybir.AxisListType

# rows with (r % 16) in this set compute n2 on DVE instead of ACT (load balance)
N2_DVE = set(range(16)) - {3, 7, 15}

_cache = {}


def _norm_blocks(nc, pre, t, tag_prefix, parts):
    """l2-normalize rows of an SBUF tile [parts,128]; returns normalized tile."""
    sq = pre.tile([parts, 128], F32, tag=f"{tag_prefix}sq")
    nc.vector.tensor_mul(sq[:], t[:], t[:])
    ss = pre.tile([parts, 1], F32, tag=f"{tag_prefix}ss")
    nc.vector.reduce_sum(ss[:], sq[:], axis=AX.X)
    nrm = pre.tile([parts, 1], F32, tag=f"{tag_prefix}nrm")
    nc.scalar.sqrt(nrm[:], ss[:])
    nrm2 = pre.tile([parts, 1], F32, tag=f"{tag_prefix}nrm2")
    nc.vector.tensor_scalar_max(nrm2[:], nrm[:], 1e-12)
    rn = pre.tile([parts, 1], F32, tag=f"{tag_prefix}rn")
    nc.vector.reciprocal(rn[:], nrm2[:])
    tn = pre.tile([parts, 128], F32, tag=f"{tag_prefix}tn")
    nc.vector.tensor_scalar_mul(tn[:], t[:], rn[:])
    return tn


def _transpose_512(nc, pre, pps_tile, ident, dram_ap, normalize):
    """Load [512,128] DRAM tensor (one DMA), optionally l2-normalize rows
    (norm chain batched across the 4 row-blocks), transpose into the given
    PSUM tile [128,512]."""
    big = pre.tile([128, 512], F32, tag="ld")
    for blk in range(4):
        nc.sync.dma_start(
            big[:, blk * 128 : (blk + 1) * 128],
            dram_ap[blk * 128 : (blk + 1) * 128, :],
        )
    t = big
    if normalize:
        sq = pre.tile([128, 512], F32, tag="nsq")
        nc.vector.tensor_mul(sq[:], big[:], big[:])
        ss = pre.tile([128, 4], F32, tag="nss")
        nc.vector.reduce_sum(ss[:], sq[:].rearrange("p (a d) -> p a d", a=4), axis=AX.X)
        nrm = pre.tile([128, 4], F32, tag="nnrm")
        nc.scalar.sqrt(nrm[:], ss[:])
        nrm2 = pre.tile([128, 4], F32, tag="nnrm2")
        nc.vector.tensor_scalar_max(nrm2[:], nrm[:], 1e-12)
        rn = pre.tile([128, 4], F32, tag="nrn")
        nc.vector.reciprocal(rn[:], nrm2[:])
        tn = pre.tile([128, 512], F32, tag="ntn")
        for blk in range(4):
            nc.vector.tensor_scalar_mul(
                tn[:, blk * 128 : (blk + 1) * 128],
                big[:, blk * 128 : (blk + 1) * 128],
                rn[:, blk : blk + 1],
            )
        t = tn
    for blk in range(4):
        nc.tensor.transpose(
            pps_tile[:, blk * 128 : (blk + 1) * 128],
            t[:, blk * 128 : (blk + 1) * 128],
            ident[:],
        )


def _prep_link(nc, pre, pps, const, ident, dram_ap, i, want_pos, want_f32=False):
    """Top-3-per-column mask + column-normalize of link [d,e].
    Returns (negW bf16 [d,e], W bf16 [d,e] or None, negW fp32 or None)."""
    lt = pre.tile([128, 128], F32, tag="wld")
    nc.sync.dma_start(lt[:], dram_ap[:, :])
    tpw = pps.tile([128, 128], F32, tag="tpw")
    nc.tensor.transpose(tpw[:], lt[:], ident[:])
    wt = pre.tile([128, 128], F32, tag="wt")
    nc.scalar.copy(wt[:], tpw[:])  # [e, d]

    x = wt
    m = None
    for k in range(3):
        m = pre.tile([128, 1], F32, tag=f"wm{k}")
        nc.vector.reduce_max(m[:], x[:], axis=AX.X)
        if k < 2:
            msk = pre.tile([128, 128], F32, tag=f"wmask{k}")
            # ((x >= m) * -2) + x : push current max below everything
            nc.vector.tensor_scalar(msk[:], x[:], m[:], -2.0, op0=OP.is_ge, op1=OP.mult)
            x2 = pre.tile([128, 128], F32, tag=f"wx{k}")
            nc.vector.tensor_add(x2[:], x[:], msk[:])
            x = x2
    # m = 3rd-largest original value per row; keep entries >= m
    wm = pre.tile([128, 128], F32, tag="wkeep")
    nc.vector.scalar_tensor_tensor(wm[:], wt[:], m[:], wt[:], op0=OP.is_ge, op1=OP.mult)
    cs = pre.tile([128, 1], F32, tag="wcs")
    nc.vector.reduce_sum(cs[:], wm[:], axis=AX.X)
    cse = pre.tile([128, 1], F32, tag="wcse")
    nc.vector.tensor_scalar_add(cse[:], cs[:], 1e-8)
    rc = pre.tile([128, 1], F32, tag="wrc")
    nc.vector.reciprocal(rc[:], cse[:])
    nrc = pre.tile([128, 1], F32, tag="wnrc")
    nc.scalar.mul(nrc[:], rc[:], -1.0)
    wnT = pre.tile([128, 128], F32, tag=f"wnT{i}", name=f"wnT{i}")
    nc.vector.tensor_scalar_mul(wnT[:], wm[:], nrc[:])  # [e, d] (negated)
    tpw2 = pps.tile([128, 128], F32, tag="tpw")
    nc.tensor.transpose(tpw2[:], wnT[:], ident[:])
    negw = const.tile([128, 128], BF16, tag=f"negW{i}", name=f"negW{i}")
    nc.scalar.copy(negw[:], tpw2[:])  # [d, e] bf16, negated
    posw = None
    if want_pos:
        posw = const.tile([128, 128], BF16, tag=f"posW{i}", name=f"posW{i}")
        nc.scalar.mul(posw[:], tpw2[:], -1.0)  # [d, e] bf16, positive
    negwf = None
    if want_f32:
        negwf = const.tile([128, 128], F32, tag=f"negWf{i}", name=f"negWf{i}")
        nc.scalar.copy(negwf[:], tpw2[:])  # [d, e] fp32, negated
    return negw, posw, negwf


def _build():
    nc = bacc.Bacc("TRN2", target_bir_lowering=False, debug=False)
    de1 = [nc.dram_tensor(f"emb1_{l}", [RPC, D], F32, kind="ExternalInput") for l in range(3)]
    dc1 = [nc.dram_tensor(f"cert1_{l}", [RPC, D], F32, kind="ExternalInput") for l in (1, 2)]
    de2 = [nc.dram_tensor(f"emb2_{l}", [B2, D], F32, kind="ExternalInput") for l in range(3)]
    dc2 = [nc.dram_tensor(f"cert2_{l}", [B2, D], F32, kind="ExternalInput") for l in (1, 2)]
    dal = [nc.dram_tensor(f"alpha_{l}", [D, 1], F32, kind="ExternalInput") for l in (1, 2)]
    dbe = [nc.dram_tensor(f"beta_{l}", [D, 1], F32, kind="ExternalInput") for l in (1, 2)]
    dlk = [nc.dram_tensor(f"link_{l}", [D, D], F32, kind="ExternalInput") for l in range(2)]
    did = nc.dram_tensor("ident", [D, D], F32, kind="ExternalInput")
    dout = nc.dram_tensor("ovr", [RPC, B2], F32, kind="ExternalOutput")

    with tile.TileContext(nc) as tc:
        with tc.tile_pool(name="const", bufs=1) as const:
            ident = const.tile([128, 128], F32, tag="ident")
            nc.sync.dma_start(ident[:], did.ap())
            identb = const.tile([128, 128], BF16, tag="identb")
            nc.vector.tensor_copy(identb[:], ident[:])
            onesb = const.tile([128, 1], BF16, tag="onesb")
            nc.vector.memset(onesb[:], 1.0)
            negonesb = const.tile([128, 1], BF16, tag="negonesb")
            nc.vector.memset(negonesb[:], -1.0)
            nacol = []
            nbcol = []
            for i in range(2):
                a = const.tile([128, 1], F32, tag=f"acol{i}", name=f"acol{i}")
                nc.sync.dma_start(a[:], dal[i].ap())
                na = const.tile([128, 1], F32, tag=f"nacol{i}", name=f"nacol{i}")
                nc.scalar.mul(na[:], a[:], -1.0)
                nacol.append(na)
                b = const.tile([128, 1], F32, tag=f"bcol{i}", name=f"bcol{i}")
                nc.sync.dma_start(b[:], dbe[i].ap())
                nb = const.tile([128, 1], F32, tag=f"nbcol{i}", name=f"nbcol{i}")
                nc.scalar.mul(nb[:], b[:], -1.0)
                nbcol.append(nb)

            # e2T: l=1 fp32 (ACT in); l=0 bf16 (DVE); l=2 both (split rows)
            e2T = [None] * 3
            e2T2b = None
            c2T = [None] * 2
            e1T = [None] * 3  # l=0,2: positive; l=1: negated (ACT bias)
            ne1T2 = None  # negated l=2 (ACT bias for ACT-rows)
            nscT = [None] * 2
            with tc.tile_pool(name="pre", bufs=6) as pre, tc.tile_pool(
                name="prepsum", bufs=2, space="PSUM"
            ) as pps:
                for l in range(3):
                    tp = pps.tile([128, 512], F32, tag="tp512")
                    _transpose_512(nc, pre, tp, ident, de2[l].ap(), True)
                    dt = F32 if l == 2 else BF16
                    e2T[l] = const.tile([128, 512], dt, tag=f"e2T{l}", name=f"e2T{l}")
                    nc.scalar.copy(e2T[l][:], tp[:])
                    if l == 2:
                        e2T2b = const.tile([128, 512], BF16, tag="e2T2b")
                        nc.vector.tensor_copy(e2T2b[:], tp[:])
                for i in range(2):
                    tp = pps.tile([128, 512], F32, tag="tp512")
                    _transpose_512(nc, pre, tp, ident, dc2[i].ap(), False)
                    c2T[i] = const.tile([128, 512], BF16, tag=f"c2T{i}", name=f"c2T{i}")
                    nc.scalar.copy(c2T[i][:], tp[:])
                # emb1 shard: normalize rows, transpose -> [d, r]
                for l in range(3):
                    t = pre.tile([64, 128], F32, tag="e1ld")
                    nc.sync.dma_start(t[:], de1[l].ap())
                    tn = _norm_blocks(nc, pre, t, "e1", 64)
                    if l == 1:
                        tn2 = pre.tile([64, 128], F32, tag="e1neg")
                        nc.scalar.mul(tn2[:], tn[:], -1.0)
                        tn = tn2
                    tp64 = pps.tile([128, 64], F32, tag="tp64")
                    nc.tensor.transpose(tp64[:], tn[:], ident[:64, :64])
                    e1T[l] = const.tile([128, 64], F32, tag=f"e1T{l}", name=f"e1T{l}")
                    nc.scalar.copy(e1T[l][:], tp64[:])
                    if l == 2:
                        ne1T2 = const.tile([128, 64], F32, tag="ne1T2")
                        nc.scalar.mul(ne1T2[:], tp64[:], -1.0)
                # cert1 shard: transpose, scale by -alpha -> [d, r]
                for i in range(2):
                    t = pre.tile([64, 128], F32, tag="c1ld")
                    nc.sync.dma_start(t[:], dc1[i].ap())
                    tp64 = pps.tile([128, 64], F32, tag="tp64")
                    nc.tensor.transpose(tp64[:], t[:], ident[:64, :64])
                    c1T = pre.tile([128, 64], F32, tag="c1T")
                    nc.scalar.copy(c1T[:], tp64[:])
                    nscT[i] = const.tile([128, 64], F32, tag=f"nscT{i}", name=f"nscT{i}")
                    nc.vector.tensor_scalar_mul(nscT[i][:], c1T[:], nacol[i][:])
                negW1, _, negW1f = _prep_link(nc, pre, pps, const, ident, dlk[0].ap(), 0, False, True)
                negW2, posW2, _ = _prep_link(nc, pre, pps, const, ident, dlk[1].ap(), 1, True)
                # constants for virtualized n0: n0 = e2sq0 + a0*e2T0 + c0
                e2sqT0 = pre.tile([128, 512], F32, tag="e2sqT0", name="e2sqT0")
                nc.vector.tensor_mul(e2sqT0[:], e2T[0][:], e2T[0][:])
                tpE = pps.tile([128, 512], F32, tag="tp512")
                nc.tensor.matmul(tpE[:], lhsT=negW1f[:], rhs=e2sqT0[:], start=True, stop=True)
                negE0 = const.tile([128, 512], BF16, tag="negE0", name="negE0")
                nc.scalar.copy(negE0[:], tpE[:])
                a0T = const.tile([128, 64], F32, tag="a0T", name="a0T")
                nc.scalar.mul(a0T[:], e1T[0][:], -2.0)
                c0T = pre.tile([128, 64], F32, tag="c0T", name="c0T")
                nc.vector.tensor_mul(c0T[:], e1T[0][:], e1T[0][:])
                tpD = pps.tile([128, 64], F32, tag="tp64")
                nc.tensor.matmul(tpD[:], lhsT=negW1f[:], rhs=c0T[:], start=True, stop=True)
                negd0T = const.tile([128, 64], F32, tag="negd0T", name="negd0T")
                nc.scalar.copy(negd0T[:], tpD[:])

            with tc.tile_pool(name="row", bufs=4) as rowp, tc.tile_pool(
                name="pair", bufs=2
            ) as pairp, tc.tile_pool(name="psA", bufs=2, space="PSUM") as psA, tc.tile_pool(
                name="psB", bufs=1, space="PSUM"
            ) as psB, tc.tile_pool(name="psC", bufs=2, space="PSUM") as psC:
                C4 = None
                for rp in range(RPC // 2):
                    r0 = 2 * rp
                    if rp % 2 == 0:
                        C4 = psC.tile([128, 512], F32, tag="C4")
                    Ap = psA.tile([128, 1024], F32, tag="Ap")
                    Bpair = psB.tile([128, 1024], F32, tag="Bpair")
                    Q1p = pairp.tile([128, 1024], F32, tag="Q1p")
                    Q2p = pairp.tile([128, 1024], F32, tag="Q2p")
                    n0h = [None, None]
                    n1h = [None, None]
                    n2h = [None, None]
                    v1p = pairp.tile([128, 1024], BF16, tag="v1p")
                    v2p = pairp.tile([128, 1024], BF16, tag="v2p")
                    for h in range(2):
                        r = r0 + h
                        fo = 512 * h
                        # n0 virtualized: negV0 = a0 * (-W1); its matmul against
                        # e2T0 plus I*negE0 reproduce -W1^T n0 up to the
                        # per-partition constant negd0T handled in the v1 STT
                        negV0 = rowp.tile([128, 128], BF16, tag="negV0")
                        nc.vector.tensor_scalar_mul(
                            negV0[:], negW1[:], a0T[:, r : r + 1]
                        )
                        n0h[h] = negV0
                        # n1 on ACT
                        n1 = rowp.tile([128, 512], BF16, tag="n1")
                        nc.scalar.activation(
                            n1[:], e2T[1][:], AF.Square, bias=e1T[1][:, r : r + 1]
                        )
                        n1h[h] = n1
                        # n2: DVE on some rows, ACT on the rest
                        n2 = rowp.tile([128, 512], BF16, tag="n2")
                        if (r % 16) in N2_DVE:
                            d2 = rowp.tile([128, 512], BF16, tag="d2")
                            nc.vector.tensor_scalar_sub(
                                d2[:], e2T2b[:], e1T[2][:, r : r + 1]
                            )
                            nc.vector.tensor_mul(n2[:], d2[:], d2[:])
                        else:
                            nc.scalar.activation(
                                n2[:], e2T[2][:], AF.Square, bias=ne1T2[:, r : r + 1]
                            )
                        n2h[h] = n2
                        nc.scalar.activation(
                            Q1p[:, fo : fo + 512],
                            c2T[0][:],
                            AF.Sigmoid,
                            bias=nbcol[0][:],
                            scale=nscT[0][:, r : r + 1],
                        )
                        nc.scalar.activation(
                            Q2p[:, fo : fo + 512],
                            c2T[1][:],
                            AF.Sigmoid,
                            bias=nbcol[1][:],
                            scale=nscT[1][:, r : r + 1],
                        )
                    # matmuls interleaved across the two halves so consecutive
                    # PE ops never accumulate into the same PSUM region
                    for h in range(2):
                        nc.tensor.matmul(
                            Ap[:, 512 * h : 512 * h + 512], lhsT=n0h[h][:],
                            rhs=e2T[0][:], start=True, stop=False,
                        )
                    for h in range(2):
                        nc.tensor.matmul(
                            Ap[:, 512 * h : 512 * h + 512], lhsT=identb[:],
                            rhs=negE0[:], start=False, stop=False,
                        )
                    for h in range(2):
                        nc.tensor.matmul(
                            Ap[:, 512 * h : 512 * h + 512], lhsT=identb[:],
                            rhs=n1h[h][:], start=False, stop=True,
                        )
                    for h in range(2):
                        nc.tensor.matmul(
                            Bpair[:, 512 * h : 512 * h + 512], lhsT=negW2[:],
                            rhs=n1h[h][:], start=True, stop=False,
                        )
                    # v1 = Q1 * (A + negd0) per half (STT: scalar slot carries
                    # the per-partition constant term of -W1^T n0)
                    for h in range(2):
                        r = r0 + h
                        nc.vector.scalar_tensor_tensor(
                            v1p[:, 512 * h : 512 * h + 512],
                            Ap[:, 512 * h : 512 * h + 512],
                            negd0T[:, r : r + 1],
                            Q1p[:, 512 * h : 512 * h + 512],
                            op0=OP.add,
                            op1=OP.mult,
                        )
                    for h in range(2):
                        nc.tensor.matmul(
                            Bpair[:, 512 * h : 512 * h + 512], lhsT=posW2[:],
                            rhs=v1p[:, 512 * h : 512 * h + 512], start=False, stop=False,
                        )
                    for h in range(2):
                        nc.tensor.matmul(
                            Bpair[:, 512 * h : 512 * h + 512], lhsT=identb[:],
                            rhs=n2h[h][:], start=False, stop=True,
                        )
                    # batched v2 = Q2 * B over the pair
                    nc.vector.tensor_mul(v2p[:], Q2p[:], Bpair[:])
                    for h in range(2):
                        r = r0 + h
                        po = 32 * (r % 4)
                        nc.tensor.matmul(
                            C4[po : po + 1, :], lhsT=onesb[:], rhs=n2h[h][:],
                            start=True, stop=False, tile_position=(0, po),
                        )
                    for h in range(2):
                        r = r0 + h
                        po = 32 * (r % 4)
                        nc.tensor.matmul(
                            C4[po : po + 1, :], lhsT=negonesb[:],
                            rhs=v2p[:, 512 * h : 512 * h + 512], start=False, stop=True,
                            tile_position=(0, po),
                        )
                    if rp % 2 == 1:
                        stag = rowp.tile([128, 512], F32, tag="stag")
                        nc.vector.tensor_copy(stag[:], C4[:])
                        nc.sync.dma_start(
                            dout.ap()[r0 - 2 : r0 + 2, :], stag[:][0:97:32, :]
                        )
    nc.compile()
    return nc


def _get_nc():
    if "nc" not in _cache:
        _cache["nc"] = _build()
    return _cache["nc"]


def kernel(**inputs):
    nc = _get_nc()
    ident = np.eye(D, dtype=np.float32)
    in_maps = []
    for c in range(N_CORES):
        sl = slice(c * RPC, (c + 1) * RPC)
        m = {"ident": ident}
        for l in range(3):
            m[f"emb1_{l}"] = np.ascontiguousarray(inputs[f"emb1_{l}"][sl])
            m[f"emb2_{l}"] = np.asarray(inputs[f"emb2_{l}"])
        for l in (1, 2):
            m[f"cert1_{l}"] = np.ascontiguousarray(inputs[f"cert1_{l}"][sl])
            m[f"cert2_{l}"] = np.asarray(inputs[f"cert2_{l}"])
            m[f"alpha_{l}"] = np.asarray(inputs[f"alpha_{l}"]).reshape(D, 1)
            m[f"beta_{l}"] = np.asarray(inputs[f"beta_{l}"]).reshape(D, 1)
        for l in range(2):
            m[f"link_{l}"] = np.asarray(inputs[f"link_{l}"])
        in_maps.append(m)
    trace = bool(int(os.environ.get("AVSL_TRACE", "0")))
    res = run_bass_kernel_spmd(nc, in_maps, core_ids=list(range(N_CORES)), trace=trace)
    _cache["last_result"] = res
    return np.concatenate([res.results[c]["ovr"] for c in range(N_CORES)], axis=0)


# revision 39
# speedup vs baseline: 1.0235x; 1.0235x over previous
"""AVSL similarity kernel for Trainium2 (8 NeuronCores, data-parallel over B1).

Math (per (b1,b2) pair, d-vector chain over 3 layers):
  n_l = (normalize(emb1_l[b1]) - normalize(emb2_l[b2]))**2        [D]
  hat_0 = n_0
  hat_l = (1-P_l) * (hat_{l-1} @ W_l) + P_l * n_l,  l=1,2
  P_l   = sigmoid(alpha_l * cert1_l[b1] * cert2_l[b2] + beta_l)
  W_l   = col-top3-masked, col-normalized link_{l-1}
  out[b1,b2] = sum_d hat_2

Device decomposition, [d(=128 partitions), b2(=512 free)] layout, Q_l = 1-P_l
(sigmoid of negated argument):
  A  = n1 - W1^T n0                       (PE: negated weights + I*n1)
  v1 = Q1 * A          => hat1 = n1 - v1  (DVE; Q via ACT)
  B  = n2 - W2^T n1 + W2^T v1  (= n2 - mm2)   (PE)
  v2 = Q2 * B          => hat2 = n2 - v2  (DVE)
  out_row = 1^T n2 - 1^T v2               (PE M=1 matmuls, 4 rows per PSUM bank
                                           at partitions 0/32/64/96)
Rows processed in pairs: A/B PSUM tiles and Q/v tiles span two rows
([128,1024], 2 PSUM banks) so the PSUM-read multiplies amortize overhead.
Engine split: ACT: n1, Q1, Q2 (+ n2 on most rows) | DVE: n0 (sub+sq), v1, v2,
out-copy (+ n2 on some rows) | PE: 7 matmuls/row. Matmul operands bf16.

Sharding: emb1/cert1 rows split 64/core; emb2/cert2/links/alpha/beta replicated.
"""
import os
import sys

sys.path.insert(0, "/opt/trn_rl_repo")

import numpy as np

import concourse.bass as bass
import concourse.tile as tile
from concourse import bacc, mybir
from concourse.bass_utils import run_bass_kernel_spmd

N_CORES = 8
B1, B2, D = 512, 512, 128
RPC = B1 // N_CORES  # rows of ovr_sim per core
F32 = mybir.dt.float32
BF16 = mybir.dt.bfloat16
AF = mybir.ActivationFunctionType
OP = mybir.AluOpType
AX = mybir.AxisListType

# rows with (r % 16) in this set compute n2 on DVE instead of ACT (load balance)
N2_DVE = set(range(16)) - {3, 7, 15}

_cache = {}


def _norm_blocks(nc, pre, t, tag_prefix, parts):
    """l2-normalize rows of an SBUF tile [parts,128]; returns normalized tile."""
    sq = pre.tile([parts, 128], F32, tag=f"{tag_prefix}sq")
    nc.vector.tensor_mul(sq[:], t[:], t[:])
    ss = pre.tile([parts, 1], F32, tag=f"{tag_prefix}ss")
    nc.vector.reduce_sum(ss[:], sq[:], axis=AX.X)
    nrm = pre.tile([parts, 1], F32, tag=f"{tag_prefix}nrm")
    nc.scalar.sqrt(nrm[:], ss[:])
    nrm2 = pre.tile([parts, 1], F32, tag=f"{tag_prefix}nrm2")
    nc.vector.tensor_scalar_max(nrm2[:], nrm[:], 1e-12)
    rn = pre.tile([parts, 1], F32, tag=f"{tag_prefix}rn")
    nc.vector.reciprocal(rn[:], nrm2[:])
    tn = pre.tile([parts, 128], F32, tag=f"{tag_prefix}tn")
    nc.vector.tensor_scalar_mul(tn[:], t[:], rn[:])
    return tn


def _transpose_512(nc, pre, pps_tile, ident, dram_ap, normalize):
    """Load [512,128] DRAM tensor (one DMA), optionally l2-normalize rows
    (norm chain batched across the 4 row-blocks), transpose into the given
    PSUM tile [128,512]."""
    big = pre.tile([128, 512], F32, tag="ld")
    for blk in range(4):
        nc.sync.dma_start(
            big[:, blk * 128 : (blk + 1) * 128],
            dram_ap[blk * 128 : (blk + 1) * 128, :],
        )
    t = big
    if normalize:
        sq = pre.tile([128, 512], F32, tag="nsq")
        nc.vector.tensor_mul(sq[:], big[:], big[:])
        ss = pre.tile([128, 4], F32, tag="nss")
        nc.vector.reduce_sum(ss[:], sq[:].rearrange("p (a d) -> p a d", a=4), axis=AX.X)
        nrm = pre.tile([128, 4], F32, tag="nnrm")
        nc.scalar.sqrt(nrm[:], ss[:])
        nrm2 = pre.tile([128, 4], F32, tag="nnrm2")
        nc.vector.tensor_scalar_max(nrm2[:], nrm[:], 1e-12)
        rn = pre.tile([128, 4], F32, tag="nrn")
        nc.vector.reciprocal(rn[:], nrm2[:])
        tn = pre.tile([128, 512], F32, tag="ntn")
        for blk in range(4):
            nc.vector.tensor_scalar_mul(
                tn[:, blk * 128 : (blk + 1) * 128],
                big[:, blk * 128 : (blk + 1) * 128],
                rn[:, blk : blk + 1],
            )
        t = tn
    for blk in range(4):
        nc.tensor.transpose(
            pps_tile[:, blk * 128 : (blk + 1) * 128],
            t[:, blk * 128 : (blk + 1) * 128],
            ident[:],
        )


def _prep_link(nc, pre, pps, const, ident, dram_ap, i, want_pos, want_f32=False):
    """Top-3-per-column mask + column-normalize of link [d,e].
    Returns (negW bf16 [d,e], W bf16 [d,e] or None, negW fp32 or None)."""
    lt = pre.tile([128, 128], F32, tag="wld")
    nc.sync.dma_start(lt[:], dram_ap[:, :])
    tpw = pps.tile([128, 128], F32, tag="tpw")
    nc.tensor.transpose(tpw[:], lt[:], ident[:])
    wt = pre.tile([128, 128], F32, tag="wt")
    nc.scalar.copy(wt[:], tpw[:])  # [e, d]

    x = wt
    m = None
    for k in range(3):
        m = pre.tile([128, 1], F32, tag=f"wm{k}")
        nc.vector.reduce_max(m[:], x[:], axis=AX.X)
        if k < 2:
            msk = pre.tile([128, 128], F32, tag=f"wmask{k}")
            # ((x >= m) * -2) + x : push current max below everything
            nc.vector.tensor_scalar(msk[:], x[:], m[:], -2.0, op0=OP.is_ge, op1=OP.mult)
            x2 = pre.tile([128, 128], F32, tag=f"wx{k}")
            nc.vector.tensor_add(x2[:], x[:], msk[:])
            x = x2
    # m = 3rd-largest original value per row; keep entries >= m
    wm = pre.tile([128, 128], F32, tag="wkeep")
    nc.vector.scalar_tensor_tensor(wm[:], wt[:], m[:], wt[:], op0=OP.is_ge, op1=OP.mult)
    cs = pre.tile([128, 1], F32, tag="wcs")
    nc.vector.reduce_sum(cs[:], wm[:], axis=AX.X)
    cse = pre.tile([128, 1], F32, tag="wcse")
    nc.vector.tensor_scalar_add(cse[:], cs[:], 1e-8)
    rc = pre.tile([128, 1], F32, tag="wrc")
    nc.vector.reciprocal(rc[:], cse[:])
    nrc = pre.tile([128, 1], F32, tag="wnrc")
    nc.scalar.mul(nrc[:], rc[:], -1.0)
    wnT = pre.tile([128, 128], F32, tag=f"wnT{i}", name=f"wnT{i}")
    nc.vector.tensor_scalar_mul(wnT[:], wm[:], nrc[:])  # [e, d] (negated)
    tpw2 = pps.tile([128, 128], F32, tag="tpw")
    nc.tensor.transpose(tpw2[:], wnT[:], ident[:])
    negw = const.tile([128, 128], BF16, tag=f"negW{i}", name=f"negW{i}")
    nc.scalar.copy(negw[:], tpw2[:])  # [d, e] bf16, negated
    posw = None
    if want_pos:
        posw = const.tile([128, 128], BF16, tag=f"posW{i}", name=f"posW{i}")
        nc.scalar.mul(posw[:], tpw2[:], -1.0)  # [d, e] bf16, positive
    negwf = None
    if want_f32:
        negwf = const.tile([128, 128], F32, tag=f"negWf{i}", name=f"negWf{i}")
        nc.scalar.copy(negwf[:], tpw2[:])  # [d, e] fp32, negated
    return negw, posw, negwf


def _build():
    nc = bacc.Bacc("TRN2", target_bir_lowering=False, debug=False)
    de1 = [nc.dram_tensor(f"emb1_{l}", [RPC, D], F32, kind="ExternalInput") for l in range(3)]
    dc1 = [nc.dram_tensor(f"cert1_{l}", [RPC, D], F32, kind="ExternalInput") for l in (1, 2)]
    de2 = [nc.dram_tensor(f"emb2_{l}", [B2, D], F32, kind="ExternalInput") for l in range(3)]
    dc2 = [nc.dram_tensor(f"cert2_{l}", [B2, D], F32, kind="ExternalInput") for l in (1, 2)]
    dal = [nc.dram_tensor(f"alpha_{l}", [D, 1], F32, kind="ExternalInput") for l in (1, 2)]
    dbe = [nc.dram_tensor(f"beta_{l}", [D, 1], F32, kind="ExternalInput") for l in (1, 2)]
    dlk = [nc.dram_tensor(f"link_{l}", [D, D], F32, kind="ExternalInput") for l in range(2)]
    did = nc.dram_tensor("ident", [D, D], F32, kind="ExternalInput")
    dout = nc.dram_tensor("ovr", [RPC, B2], F32, kind="ExternalOutput")

    with tile.TileContext(nc) as tc:
        with tc.tile_pool(name="const", bufs=1) as const:
            ident = const.tile([128, 128], F32, tag="ident")
            nc.sync.dma_start(ident[:], did.ap())
            identb = const.tile([128, 128], BF16, tag="identb")
            nc.vector.tensor_copy(identb[:], ident[:])
            onesb = const.tile([128, 1], BF16, tag="onesb")
            nc.vector.memset(onesb[:], 1.0)
            negonesb = const.tile([128, 1], BF16, tag="negonesb")
            nc.vector.memset(negonesb[:], -1.0)
            nacol = []
            nbcol = []
            for i in range(2):
                a = const.tile([128, 1], F32, tag=f"acol{i}", name=f"acol{i}")
                nc.sync.dma_start(a[:], dal[i].ap())
                na = const.tile([128, 1], F32, tag=f"nacol{i}", name=f"nacol{i}")
                nc.scalar.mul(na[:], a[:], -1.0)
                nacol.append(na)
                b = const.tile([128, 1], F32, tag=f"bcol{i}", name=f"bcol{i}")
                nc.sync.dma_start(b[:], dbe[i].ap())
                nb = const.tile([128, 1], F32, tag=f"nbcol{i}", name=f"nbcol{i}")
                nc.scalar.mul(nb[:], b[:], -1.0)
                nbcol.append(nb)

            # e2T: l=1 fp32 (ACT in); l=0 bf16 (DVE); l=2 both (split rows)
            e2T = [None] * 3
            e2T2b = None
            c2T = [None] * 2
            e1T = [None] * 3  # l=0,2: positive; l=1: negated (ACT bias)
            ne1T2 = None  # negated l=2 (ACT bias for ACT-rows)
            nscT = [None] * 2
            with tc.tile_pool(name="pre", bufs=4) as pre, tc.tile_pool(
                name="prepsum", bufs=2, space="PSUM"
            ) as pps:
                for l in range(3):
                    tp = pps.tile([128, 512], F32, tag="tp512")
                    _transpose_512(nc, pre, tp, ident, de2[l].ap(), True)
                    dt = F32 if l == 2 else BF16
                    e2T[l] = const.tile([128, 512], dt, tag=f"e2T{l}", name=f"e2T{l}")
                    nc.scalar.copy(e2T[l][:], tp[:])
                    if l == 2:
                        e2T2b = const.tile([128, 512], BF16, tag="e2T2b")
                        nc.vector.tensor_copy(e2T2b[:], tp[:])
                for i in range(2):
                    tp = pps.tile([128, 512], F32, tag="tp512")
                    _transpose_512(nc, pre, tp, ident, dc2[i].ap(), False)
                    c2T[i] = const.tile([128, 512], BF16, tag=f"c2T{i}", name=f"c2T{i}")
                    nc.scalar.copy(c2T[i][:], tp[:])
                # emb1 shard: normalize rows, transpose -> [d, r]
                for l in range(3):
                    t = pre.tile([64, 128], F32, tag="e1ld")
                    nc.sync.dma_start(t[:], de1[l].ap())
                    tn = _norm_blocks(nc, pre, t, "e1", 64)
                    if l == 1:
                        tn2 = pre.tile([64, 128], F32, tag="e1neg")
                        nc.scalar.mul(tn2[:], tn[:], -1.0)
                        tn = tn2
                    tp64 = pps.tile([128, 64], F32, tag="tp64")
                    nc.tensor.transpose(tp64[:], tn[:], ident[:64, :64])
                    e1T[l] = const.tile([128, 64], F32, tag=f"e1T{l}", name=f"e1T{l}")
                    nc.scalar.copy(e1T[l][:], tp64[:])
                    if l == 2:
                        ne1T2 = const.tile([128, 64], F32, tag="ne1T2")
                        nc.scalar.mul(ne1T2[:], tp64[:], -1.0)
                # cert1 shard: transpose, scale by -alpha -> [d, r]
                for i in range(2):
                    t = pre.tile([64, 128], F32, tag="c1ld")
                    nc.sync.dma_start(t[:], dc1[i].ap())
                    tp64 = pps.tile([128, 64], F32, tag="tp64")
                    nc.tensor.transpose(tp64[:], t[:], ident[:64, :64])
                    c1T = pre.tile([128, 64], F32, tag="c1T")
                    nc.scalar.copy(c1T[:], tp64[:])
                    nscT[i] = const.tile([128, 64], F32, tag=f"nscT{i}", name=f"nscT{i}")
                    nc.vector.tensor_scalar_mul(nscT[i][:], c1T[:], nacol[i][:])
                negW1, _, negW1f = _prep_link(nc, pre, pps, const, ident, dlk[0].ap(), 0, False, True)
                negW2, posW2, _ = _prep_link(nc, pre, pps, const, ident, dlk[1].ap(), 1, True)
                # constants for virtualized n0: n0 = e2sq0 + a0*e2T0 + c0
                e2sqT0 = pre.tile([128, 512], F32, tag="e2sqT0", name="e2sqT0")
                nc.vector.tensor_mul(e2sqT0[:], e2T[0][:], e2T[0][:])
                tpE = pps.tile([128, 512], F32, tag="tp512")
                nc.tensor.matmul(tpE[:], lhsT=negW1f[:], rhs=e2sqT0[:], start=True, stop=True)
                negE0 = const.tile([128, 512], BF16, tag="negE0", name="negE0")
                nc.scalar.copy(negE0[:], tpE[:])
                a0T = const.tile([128, 64], F32, tag="a0T", name="a0T")
                nc.scalar.mul(a0T[:], e1T[0][:], -2.0)
                c0T = pre.tile([128, 64], F32, tag="c0T", name="c0T")
                nc.vector.tensor_mul(c0T[:], e1T[0][:], e1T[0][:])
                tpD = pps.tile([128, 64], F32, tag="tp64")
                nc.tensor.matmul(tpD[:], lhsT=negW1f[:], rhs=c0T[:], start=True, stop=True)
                negd0T = const.tile([128, 64], F32, tag="negd0T", name="negd0T")
                nc.scalar.copy(negd0T[:], tpD[:])

            with tc.tile_pool(name="row", bufs=4) as rowp, tc.tile_pool(
                name="pair", bufs=2
            ) as pairp, tc.tile_pool(name="psA", bufs=2, space="PSUM") as psA, tc.tile_pool(
                name="psB", bufs=1, space="PSUM"
            ) as psB, tc.tile_pool(name="psC", bufs=2, space="PSUM") as psC:
                C4 = None
                for rp in range(RPC // 2):
                    r0 = 2 * rp
                    if rp % 2 == 0:
                        C4 = psC.tile([128, 512], F32, tag="C4")
                    Ap = psA.tile([128, 1024], F32, tag="Ap")
                    Bpair = psB.tile([128, 1024], F32, tag="Bpair")
                    Q1p = pairp.tile([128, 1024], F32, tag="Q1p")
                    Q2p = pairp.tile([128, 1024], F32, tag="Q2p")
                    n0h = [None, None]
                    n1h = [None, None]
                    n2h = [None, None]
                    v1p = pairp.tile([128, 1024], BF16, tag="v1p")
                    v2p = pairp.tile([128, 1024], BF16, tag="v2p")
                    for h in range(2):
                        r = r0 + h
                        fo = 512 * h
                        # n0 virtualized: negV0 = a0 * (-W1); its matmul against
                        # e2T0 plus I*negE0 reproduce -W1^T n0 up to the
                        # per-partition constant negd0T handled in the v1 STT
                        negV0 = rowp.tile([128, 128], BF16, tag="negV0")
                        nc.vector.tensor_scalar_mul(
                            negV0[:], negW1[:], a0T[:, r : r + 1]
                        )
                        n0h[h] = negV0
                        # n1 on ACT
                        n1 = rowp.tile([128, 512], BF16, tag="n1")
                        nc.scalar.activation(
                            n1[:], e2T[1][:], AF.Square, bias=e1T[1][:, r : r + 1]
                        )
                        n1h[h] = n1
                        # n2: DVE on some rows, ACT on the rest
                        n2 = rowp.tile([128, 512], BF16, tag="n2")
                        if (r % 16) in N2_DVE:
                            d2 = rowp.tile([128, 512], BF16, tag="d2")
                            nc.vector.tensor_scalar_sub(
                                d2[:], e2T2b[:], e1T[2][:, r : r + 1]
                            )
                            nc.vector.tensor_mul(n2[:], d2[:], d2[:])
                        else:
                            nc.scalar.activation(
                                n2[:], e2T[2][:], AF.Square, bias=ne1T2[:, r : r + 1]
                            )
                        n2h[h] = n2
                        nc.scalar.activation(
                            Q1p[:, fo : fo + 512],
                            c2T[0][:],
                            AF.Sigmoid,
                            bias=nbcol[0][:],
                            scale=nscT[0][:, r : r + 1],
                        )
                        nc.scalar.activation(
                            Q2p[:, fo : fo + 512],
                            c2T[1][:],
                            AF.Sigmoid,
                            bias=nbcol[1][:],
                            scale=nscT[1][:, r : r + 1],
                        )
                    # matmuls interleaved across the two halves so consecutive
                    # PE ops never accumulate into the same PSUM region
                    for h in range(2):
                        nc.tensor.matmul(
                            Ap[:, 512 * h : 512 * h + 512], lhsT=n0h[h][:],
                            rhs=e2T[0][:], start=True, stop=False,
                        )
                    for h in range(2):
                        nc.tensor.matmul(
                            Ap[:, 512 * h : 512 * h + 512], lhsT=identb[:],
                            rhs=negE0[:], start=False, stop=False,
                        )
                    for h in range(2):
                        nc.tensor.matmul(
                            Ap[:, 512 * h : 512 * h + 512], lhsT=identb[:],
                            rhs=n1h[h][:], start=False, stop=True,
                        )
                    for h in range(2):
                        nc.tensor.matmul(
                            Bpair[:, 512 * h : 512 * h + 512], lhsT=negW2[:],
                            rhs=n1h[h][:], start=True, stop=False,
                        )
                    # v1 = Q1 * (A + negd0) per half (STT: scalar slot carries
                    # the per-partition constant term of -W1^T n0)
                    for h in range(2):
                        r = r0 + h
                        nc.vector.scalar_tensor_tensor(
                            v1p[:, 512 * h : 512 * h + 512],
                            Ap[:, 512 * h : 512 * h + 512],
                            negd0T[:, r : r + 1],
                            Q1p[:, 512 * h : 512 * h + 512],
                            op0=OP.add,
                            op1=OP.mult,
                        )
                    for h in range(2):
                        nc.tensor.matmul(
                            Bpair[:, 512 * h : 512 * h + 512], lhsT=posW2[:],
                            rhs=v1p[:, 512 * h : 512 * h + 512], start=False, stop=False,
                        )
                    for h in range(2):
                        nc.tensor.matmul(
                            Bpair[:, 512 * h : 512 * h + 512], lhsT=identb[:],
                            rhs=n2h[h][:], start=False, stop=True,
                        )
                    # batched v2 = Q2 * B over the pair
                    nc.vector.tensor_mul(v2p[:], Q2p[:], Bpair[:])
                    for h in range(2):
                        r = r0 + h
                        po = 32 * (r % 4)
                        nc.tensor.matmul(
                            C4[po : po + 1, :], lhsT=onesb[:], rhs=n2h[h][:],
                            start=True, stop=False, tile_position=(0, po),
                        )
                    for h in range(2):
                        r = r0 + h
                        po = 32 * (r % 4)
                        nc.tensor.matmul(
                            C4[po : po + 1, :], lhsT=negonesb[:],
                            rhs=v2p[:, 512 * h : 512 * h + 512], start=False, stop=True,
                            tile_position=(0, po),
                        )
                    if rp % 2 == 1:
                        stag = rowp.tile([128, 512], F32, tag="stag")
                        nc.vector.tensor_copy(stag[:], C4[:])
                        nc.sync.dma_start(
                            dout.ap()[r0 - 2 : r0 + 2, :], stag[:][0:97:32, :]
                        )
    nc.compile()
    return nc


def _get_nc():
    if "nc" not in _cache:
        _cache["nc"] = _build()
    return _cache["nc"]


def kernel(**inputs):
    nc = _get_nc()
    ident = np.eye(D, dtype=np.float32)
    in_maps = []
    for c in range(N_CORES):
        sl = slice(c * RPC, (c + 1) * RPC)
        m = {"ident": ident}
        for l in range(3):
            m[f"emb1_{l}"] = np.ascontiguousarray(inputs[f"emb1_{l}"][sl])
            m[f"emb2_{l}"] = np.asarray(inputs[f"emb2_{l}"])
        for l in (1, 2):
            m[f"cert1_{l}"] = np.ascontiguousarray(inputs[f"cert1_{l}"][sl])
            m[f"cert2_{l}"] = np.asarray(inputs[f"cert2_{l}"])
            m[f"alpha_{l}"] = np.asarray(inputs[f"alpha_{l}"]).reshape(D, 1)
            m[f"beta_{l}"] = np.asarray(inputs[f"beta_{l}"]).reshape(D, 1)
        for l in range(2):
            m[f"link_{l}"] = np.asarray(inputs[f"link_{l}"])
        in_maps.append(m)
    trace = bool(int(os.environ.get("AVSL_TRACE", "0")))
    res = run_bass_kernel_spmd(nc, in_maps, core_ids=list(range(N_CORES)), trace=trace)
    _cache["last_result"] = res
    return np.concatenate([res.results[c]["ovr"] for c in range(N_CORES)], axis=0)
